# revision 1
# baseline (speedup 1.0000x reference)
"""ConformerBlock Trainium2 kernel.

Sharding: data-parallel over batch. B=16 -> 2 batch elements per core x 8 cores.
Weights replicated, no collectives.

Layout: feature-major activations xT [D, N] (host pre-transposes x / transposes
the output back). LN gamma/beta are folded into adjacent weights host-side; BN
is folded into the depthwise conv. Rel-pos attention computes S = q @
flip(rel_emb)^T, stages it in DRAM (bf16) and reads the [n, 511-n+r] window
back with a skewed access pattern. Softmax needs no max subtraction (logits
are ~0.2 std). Residual stream stays fp32/f32r; big matmuls run float32r
(1 cyc/row); attention probs, FF hidden, and conv path run bf16.
"""
import numpy as np
import ml_dtypes

import concourse.bass as bass
import concourse.bacc as bacc
import concourse.tile as tile
from concourse import mybir
from concourse.bass_utils import run_bass_kernel_spmd
from concourse.masks import make_identity

F32 = mybir.dt.float32
F32R = mybir.dt.float32r
BF16 = mybir.dt.bfloat16
AF = mybir.ActivationFunctionType
OP = mybir.AluOpType
BFNP = ml_dtypes.bfloat16

DEBUG_TAPS = False
B, NT, D = 16, 512, 512
H, DH = 8, 64
INNER = H * DH                    # 512
FF = 4 * D                        # 2048
CI = 2 * D                        # 1024
KW = 31
EPS = 1e-5
P = 128
NCORES = 8
BPC = B // NCORES                 # 2
KD = D // P                       # 4
KF = FF // P                      # 16
KC = CI // P                      # 8
RW = 1024


# --------------------------------------------------------------------------
# host-side weight folding
# --------------------------------------------------------------------------

def _host_prepare(inp):
    g = {k: np.ascontiguousarray(np.asarray(v, np.float32)) for k, v in inp.items()}
    p = {}
    scale = DH ** (-0.5)

    p['w_ff1_1'] = g['ff1_w1'] * g['ff1_ln_g'][:, None]
    p['b_ff1_1'] = g['ff1_b1'] + g['ff1_ln_b'] @ g['ff1_w1']
    p['w_ff1_2'] = 0.5 * g['ff1_w2']
    p['b_ff1_2'] = 0.5 * g['ff1_b2']
    p['w_ff2_1'] = g['ff2_w1'] * g['ff2_ln_g'][:, None]
    p['b_ff2_1'] = g['ff2_b1'] + g['ff2_ln_b'] @ g['ff2_w1']
    p['w_ff2_2'] = 0.5 * g['ff2_w2']
    p['b_ff2_2'] = 0.5 * g['ff2_b2']

    qkv_w = g['qkv_w'] * g['attn_ln_g'][:, None]
    qkv_b = g['attn_ln_b'] @ g['qkv_w']
    qkv_w[:, :INNER] *= scale
    qkv_b[:INNER] *= scale
    p['qkv_w'] = qkv_w
    p['qkv_b'] = qkv_b
    p['qkv_bv'] = np.ascontiguousarray(qkv_b[2 * INNER:][None, :])
    p['out_w'] = g['out_w']
    rm = g['rel_emb'][:1024][::-1].T                       # [DH, 1024]
    p['r_mat'] = np.ascontiguousarray(np.concatenate([rm, rm], 0))  # [128, RW]

    pw1 = g['pw1_w'] * g['conv_ln_g'][None, :]
    p['pw1_wT'] = np.ascontiguousarray(pw1.T)
    p['b_pw1'] = g['pw1_b'] + pw1 @ g['conv_ln_b']
    bnsc = g['bn_g'] / np.sqrt(g['bn_var'] + EPS)
    p['dw_w'] = g['dw_w'][:, 0, :] * bnsc[:, None]
    p['dw_b'] = (g['dw_b'] - g['bn_mean']) * bnsc + g['bn_b']
    p['pw2_wT'] = np.ascontiguousarray(g['pw2_w'].T)
    p['b_pw2'] = g['pw2_b']
    p['post_g'] = g['post_ln_g']
    p['post_b'] = g['post_ln_b']
    return p


def _col(v, nch):
    return np.ascontiguousarray(v.reshape(nch, P).T)


# --------------------------------------------------------------------------
# device program
# --------------------------------------------------------------------------

def _build_nc():
    nc = bacc.Bacc("TRN2", target_bir_lowering=False, debug=False, num_devices=1)

    def par(name, shape, dt=F32R, out=False):
        return nc.dram_tensor(name, list(shape), dt,
                              kind="ExternalOutput" if out else "ExternalInput").ap()

    pr = {}
    pr["xT"] = par("xT", [BPC, D, NT])
    pr["yO"] = par("y", [BPC, D, NT], F32, out=True)
    pr["w_ff1_1"] = par("w_ff1_1", [D, FF])
    pr["w_ff1_2"] = par("w_ff1_2", [FF, D], BF16)
    pr["w_ff2_1"] = par("w_ff2_1", [D, FF])
    pr["w_ff2_2"] = par("w_ff2_2", [FF, D], BF16)
    pr["qkv_w"] = par("qkv_w", [D, 3 * INNER])
    pr["out_w"] = par("out_w", [INNER, D])
    pr["r_mat"] = par("r_mat", [P, RW], BF16)
    pr["pw1_wT"] = par("pw1_wT", [D, 2 * CI])
    pr["pw2_wT"] = par("pw2_wT", [CI, D], BF16)
    pr["qkv_bv"] = par("qkv_bv", [1, INNER])
    pr["b_ff1_1"] = par("b_ff1_1", [P, KF], F32)
    pr["b_ff1_2"] = par("b_ff1_2", [P, KD], F32)
    pr["b_ff2_1"] = par("b_ff2_1", [P, KF], F32)
    pr["b_ff2_2"] = par("b_ff2_2", [P, KD], F32)
    pr["qkv_b"] = par("qkv_b", [P, 8], F32)
    pr["b_pw1"] = par("b_pw1", [P, 2 * KC], F32)
    pr["dw_w"] = par("dw_w", [P, KC, KW], F32)
    pr["dw_b"] = par("dw_b", [P, KC], F32)
    pr["b_pw2"] = par("b_pw2", [P, KD], F32)
    pr["post_g"] = par("post_g", [P, KD], F32)
    pr["post_b"] = par("post_b", [P, KD], F32)
    if DEBUG_TAPS:
        for i in range(1, 5):
            pr[f"dbg{i}"] = par(f"dbg{i}", [BPC, D, NT], F32, out=True)

    with tile.TileContext(nc) as tc:
        _emit(nc, tc, pr)
    nc.compile()
    return nc


def _emit(nc, tc, pr):
    from contextlib import ExitStack
    ctx = ExitStack()
    with ctx:
        sing = ctx.enter_context(tc.tile_pool(name="sing", bufs=1))
        sb = ctx.enter_context(tc.tile_pool(name="sb", bufs=2))
        ps_ = ctx.enter_context(tc.tile_pool(name="ps", bufs=1, space="PSUM"))
        dram = ctx.enter_context(tc.tile_pool(name="dram", bufs=3, space="DRAM"))

        def st(shape, dt, tag, bufs, name):
            return sb.tile(list(shape), dt, tag=tag, bufs=bufs, name=name)

        def pt(shape, dt, tag, bufs, name):
            return ps_.tile(list(shape), dt, tag=tag, bufs=bufs, name=name)

        # ---- constants ----
        ident_bf = sing.tile([P, P], BF16)
        make_identity(nc, ident_bf)
        ones1_f = sing.tile([1, P], F32)
        nc.vector.memset(ones1_f, 1.0)
        ones1_r = sing.tile([1, P], F32R)
        nc.vector.tensor_copy(ones1_r, ones1_f)
        onesD_f = sing.tile([P, 1], F32)
        nc.vector.memset(onesD_f, 1.0 / D)
        onesD_r = sing.tile([P, 1], F32R)
        nc.vector.tensor_copy(onesD_r, onesD_f)
        eps_t = sing.tile([1, 1], F32)
        nc.vector.memset(eps_t, EPS)
        zero16 = sing.tile([P, 16], F32)
        nc.vector.memset(zero16, 0.0)

        def load_small(name, shape, dt=F32):
            t = sing.tile(list(shape), dt, name=f"sb_{name}")
            nc.sync.dma_start(t[:], pr[name][:])
            return t

        sb_bff11 = load_small("b_ff1_1", [P, KF])
        sb_bff12 = load_small("b_ff1_2", [P, KD])
        sb_bff21 = load_small("b_ff2_1", [P, KF])
        sb_bff22 = load_small("b_ff2_2", [P, KD])
        sb_qkvb = load_small("qkv_b", [P, 8])
        sb_qkvbv = load_small("qkv_bv", [1, INNER], F32R)
        sb_bpw1 = load_small("b_pw1", [P, 2 * KC])
        sb_dww = load_small("dw_w", [P, KC, KW])
        sb_dwb = load_small("dw_b", [P, KC])
        sb_bpw2 = load_small("b_pw2", [P, KD])
        sb_postg = load_small("post_g", [P, KD])
        sb_postb = load_small("post_b", [P, KD])
        sb_rmat = load_small("r_mat", [P, RW], BF16)

        def load_w(ap, ktiles, fdim, dt, tag, name):
            t = st([P, ktiles, fdim], dt, tag, 1, name)
            src = ap.rearrange("(k p) f -> p k f", p=P)
            for k in range(ktiles):
                nc.scalar.dma_start(t[:, k, :], src[:, k, :])
            return t

        # ---- input ----
        x = {}
        for b in range(BPC):
            tiles = []
            for k in range(KD):
                t = st([P, NT], F32R, "xcur", 12, f"x0_{b}_{k}")
                nc.sync.dma_start(t[:], pr["xT"][b, k * P:(k + 1) * P, :])
                tiles.append(t)
            x[b] = tiles

        # ---- layernorm ----
        def ln_stats(pfx):
            prs = []
            for b in range(BPC):
                mean_ps = pt([1, NT], F32, "mm", 5, f"mean{pfx}{b}")
                for k in range(KD):
                    nc.tensor.matmul(mean_ps, onesD_r, x[b][k],
                                     start=(k == 0), stop=(k == KD - 1))
                sq = []
                for k in range(KD):
                    s = st([P, NT], F32R, "sq", 3, f"sq{pfx}{b}_{k}")
                    nc.vector.tensor_mul(s, x[b][k].bitcast(F32),
                                         x[b][k].bitcast(F32))
                    sq.append(s)
                ex2_ps = pt([1, NT], F32, "mm", 5, f"ex2{pfx}{b}")
                for k in range(KD):
                    nc.tensor.matmul(ex2_ps, onesD_r, sq[k],
                                     start=(k == 0), stop=(k == KD - 1))
                m2 = st([1, NT], F32, "sm", 6, f"m2{pfx}{b}")
                nc.scalar.activation(m2, mean_ps, AF.Square)
                var = st([1, NT], F32, "sm", 6, f"var{pfx}{b}")
                nc.vector.tensor_tensor(var, ex2_ps, m2, OP.subtract)
                sd = st([1, NT], F32, "sm", 6, f"sd{pfx}{b}")
                nc.scalar.activation(sd, var, AF.Sqrt, bias=eps_t)
                p_row = st([1, NT], F32, "sm", 6, f"prow{pfx}{b}")
                nc.vector.reciprocal(p_row, sd)
                p_r = st([1, NT], F32R, "sm", 6, f"pr{pfx}{b}")
                nc.vector.tensor_copy(p_r, p_row)
                q_r = st([1, NT], F32R, "sm", 6, f"qr{pfx}{b}")
                nc.vector.tensor_mul(q_r, mean_ps, p_row)
                prs.append((p_r, q_r))
            return prs

        def ln_bcast(b, pq, pfx):
            p_r, q_r = pq[b]
            pb = pt([P, NT], F32, "mm", 5, f"pbc{pfx}{b}")
            nc.tensor.matmul(pb, ones1_r, p_r, start=True, stop=True)
            qb = pt([P, NT], F32, "mm", 5, f"qbc{pfx}{b}")
            nc.tensor.matmul(qb, ones1_r, q_r, start=True, stop=True)
            return pb, qb

        def ln_apply(b, pq, pfx):
            pb, qb = ln_bcast(b, pq, pfx)
            hs = []
            for k in range(KD):
                tmp = st([P, NT], F32, "t2k", 3, f"t{pfx}{b}_{k}")
                nc.vector.tensor_mul(tmp, x[b][k].bitcast(F32), pb)
                h = st([P, NT], F32R, "h", 4, f"h{pfx}{b}_{k}")
                nc.vector.tensor_tensor(h, tmp, qb, OP.subtract)
                hs.append(h)
            return hs

        # ---- feed-forward ----
        def ff_stage(w1name, w2name, b1t, b2t, pfx):
            w1 = load_w(pr[w1name], KD, FF, F32R, "wbig", f"w1{pfx}")
            w2 = load_w(pr[w2name], KF, D, BF16, "wmid", f"w2{pfx}")
            pq = ln_stats(pfx)
            for b in range(BPC):
                hs = ln_apply(b, pq, pfx)
                y1 = st([P, KF, NT], BF16, "y1s", 1, f"y1s{pfx}{b}")
                for f in range(KF):
                    ps = pt([P, NT], F32, "mm", 5, f"ps1{pfx}{b}_{f}")
                    for k in range(KD):
                        nc.tensor.matmul(ps, w1[:, k, f * P:(f + 1) * P], hs[k],
                                         start=(k == 0), stop=(k == KD - 1))
                    nc.scalar.activation(y1[:, f, :], ps, AF.Silu,
                                         bias=b1t[:, f:f + 1])
                newx = []
                for f in range(KD):
                    ps = pt([P, NT], F32, "mm", 5, f"ps2{pfx}{b}_{f}")
                    for k in range(KF):
                        nc.tensor.matmul(ps, w2[:, k, f * P:(f + 1) * P],
                                         y1[:, k, :],
                                         start=(k == 0), stop=(k == KF - 1))
                    nx = st([P, NT], F32R, "xcur", 12, f"x{pfx}{b}_{f}")
                    nc.vector.scalar_tensor_tensor(
                        nx, ps, b2t[:, f:f + 1], x[b][f].bitcast(F32),
                        OP.add, OP.add)
                    newx.append(nx)
                x[b] = newx

        # ---- attention ----
        def attn_stage():
            wq = load_w(pr["qkv_w"], KD, 3 * INNER, F32R, "wbig", "wqkv")
            wo = load_w(pr["out_w"], KD, D, F32R, "wmid", "wout")
            pq = ln_stats("at")
            for b in range(BPC):
                hs = ln_apply(b, pq, "at")
                qk = []
                for f in range(8):
                    ps = pt([P, NT], F32, "mm", 5, f"qkps{b}_{f}")
                    for k in range(KD):
                        nc.tensor.matmul(ps, wq[:, k, f * P:(f + 1) * P], hs[k],
                                         start=(k == 0), stop=(k == KD - 1))
                    t = st([P, NT], BF16, "qk", 8, f"qk{b}_{f}")
                    nc.vector.tensor_scalar(t, ps, sb_qkvb[:, f:f + 1], None,
                                            OP.add)
                    qk.append(t)
                vt = []
                for n in range(KD):
                    ps = pt([P, INNER], F32, "mm", 5, f"vps{b}_{n}")
                    for k in range(KD):
                        nc.tensor.matmul(ps, hs[k][:, n * P:(n + 1) * P],
                                         wq[:, k, 2 * INNER:3 * INNER],
                                         start=(k == 0), stop=False)
                    nc.tensor.matmul(ps, ones1_r[0:1, :], sb_qkvbv,
                                     start=False, stop=True)
                    t = st([P, INNER], BF16, "vt", 4, f"vt{b}_{n}")
                    nc.any.tensor_copy(t, ps)
                    vt.append(t)
                ao = [st([P, NT], F32R, "ao", 4, f"ao{b}_{f}")
                      for f in range(KD)]
                opsbig = {}
                for h in range(H):
                    qt = qk[h // 2]
                    kt = qk[4 + h // 2]
                    po = (h % 2) * DH
                    Sd = dram.tile([NT, RW], BF16, tag="Sd", name=f"Sd{b}_{h}")
                    dots = []
                    for mi in range(KD):
                        sps = pt([P, RW], F32, "S", 1, f"sps{b}_{h}_{mi}")
                        nc.tensor.matmul(sps[:, :512],
                                         qt[po:po + DH, mi * P:(mi + 1) * P],
                                         sb_rmat[po:po + DH, :512],
                                         start=True, stop=True)
                        nc.tensor.matmul(sps[:, 512:],
                                         qt[po:po + DH, mi * P:(mi + 1) * P],
                                         sb_rmat[po:po + DH, 512:],
                                         start=True, stop=True)
                        sbf = st([P, RW], BF16, "sbf", 2, f"sbf{b}_{h}_{mi}")
                        nc.any.tensor_copy(sbf, sps)
                        nc.sync.dma_start(Sd[mi * P:(mi + 1) * P, :], sbf[:])
                        dp = pt([P, NT], F32, "mm", 5, f"dots{b}_{h}_{mi}")
                        nc.tensor.matmul(dp, qt[po:po + DH, mi * P:(mi + 1) * P],
                                         kt[po:po + DH, :], start=True, stop=True)
                        dots.append(dp)
                    attn = st([P, KD, NT], BF16, "attn", 1, f"at{b}_{h}")
                    sums = st([P, KD], F32, "sums", 2, f"sums{b}_{h}")
                    for mi in range(KD):
                        pos = st([P, NT], BF16, "pos", 2, f"pos{b}_{h}_{mi}")
                        skew = bass.AP(tensor=Sd.tensor,
                                       offset=Sd.offset + mi * P * (RW - 1) + 511,
                                       ap=[[RW - 1, P], [1, NT]])
                        nc.sync.dma_start(pos[:], skew)
                        sc = st([P, NT], BF16, "sc", 2, f"sc{b}_{h}_{mi}")
                        nc.vector.tensor_tensor(sc, dots[mi], pos, OP.add)
                        nc.scalar.activation(attn[:, mi, :], sc, AF.Exp,
                                             accum_out=sums[:, mi:mi + 1])
                    rec = st([P, KD], F32, "sums", 2, f"rec{b}_{h}")
                    nc.vector.reciprocal(rec, sums)
                    for mi in range(KD):
                        nc.vector.tensor_scalar_mul(attn[:, mi, :],
                                                    attn[:, mi, :],
                                                    rec[:, mi:mi + 1])
                    if h % 2 == 0:
                        opsb = pt([P, NT], F32, "mm", 5, f"ops{b}_{h}")
                        opsbig[h // 2] = opsb
                    else:
                        opsb = opsbig[h // 2]
                    for ki in range(KD):
                        tps = pt([P, NT], BF16, "tr", 1, f"tr{b}_{h}_{ki}")
                        for mi in range(KD):
                            nc.tensor.transpose(tps[:, mi * P:(mi + 1) * P],
                                                attn[:, mi, ki * P:(ki + 1) * P],
                                                ident_bf)
                        att = st([P, NT], BF16, "attT", 2, f"attT{b}_{h}_{ki}")
                        nc.any.tensor_copy(att, tps)
                        nc.tensor.matmul(opsb[po:po + DH, :],
                                         vt[ki][:, h * DH:(h + 1) * DH], att,
                                         start=(ki == 0), stop=(ki == KD - 1))
                    nc.any.tensor_copy(ao[h // 2][po:po + DH, :],
                                       opsb[po:po + DH, :])
                newx = []
                for f in range(KD):
                    ps = pt([P, NT], F32, "mm", 5, f"oproj{b}_{f}")
                    for k in range(KD):
                        nc.tensor.matmul(ps, wo[:, k, f * P:(f + 1) * P], ao[k],
                                         start=(k == 0), stop=(k == KD - 1))
                    nx = st([P, NT], F32R, "xcur", 12, f"xat{b}_{f}")
                    nc.vector.tensor_tensor(nx, ps, x[b][f].bitcast(F32), OP.add)
                    newx.append(nx)
                x[b] = newx

        # ---- conv module ----
        def conv_stage():
            w1 = load_w(pr["pw1_wT"], KD, 2 * CI, F32R, "wbig", "wpw1")
            w2 = load_w(pr["pw2_wT"], KC, D, BF16, "wmid", "wpw2")
            pq = ln_stats("cv")
            hg = {}
            for b in range(BPC):
                hs = ln_apply(b, pq, "cv")
                hgb = st([P, KC, NT + 30], BF16, "hglu", 2, f"hglu{b}")
                for c in range(KC):
                    nc.vector.tensor_copy(hgb[:, c, 0:15], zero16[:, 0:15])
                    nc.vector.tensor_copy(hgb[:, c, NT + 15:], zero16[:, 0:15])
                    pso = pt([P, NT], F32, "mm", 5, f"glo{b}_{c}")
                    for k in range(KD):
                        nc.tensor.matmul(pso, w1[:, k, c * P:(c + 1) * P], hs[k],
                                         start=(k == 0), stop=(k == KD - 1))
                    psg = pt([P, NT], F32, "mm", 5, f"glg{b}_{c}")
                    for k in range(KD):
                        nc.tensor.matmul(psg,
                                         w1[:, k, CI + c * P:CI + (c + 1) * P],
                                         hs[k],
                                         start=(k == 0), stop=(k == KD - 1))
                    sg = st([P, NT], F32, "t2k", 3, f"sig{b}_{c}")
                    nc.scalar.activation(sg, psg, AF.Sigmoid,
                                         bias=sb_bpw1[:, KC + c:KC + c + 1])
                    nc.vector.scalar_tensor_tensor(
                        hgb[:, c, 15:NT + 15], pso, sb_bpw1[:, c:c + 1], sg,
                        OP.add, OP.mult)
                hg[b] = hgb
            hc = {b: st([P, KC, NT], BF16, "hc", 2, f"hc{b}")
                  for b in range(BPC)}
            for c in range(KC):
                diags = []
                for k in range(KW):
                    d = st([P, P], BF16, "diag", 33, f"dg{c}_{k}")
                    nc.vector.tensor_scalar_mul(d, ident_bf,
                                                sb_dww[:, c, k:k + 1])
                    diags.append(d)
                for b in range(BPC):
                    ps = pt([P, NT], F32, "mm", 5, f"cv{b}_{c}")
                    for k in range(KW):
                        nc.tensor.matmul(ps, diags[k], hg[b][:, c, k:k + NT],
                                         start=(k == 0), stop=(k == KW - 1))
                    nc.scalar.activation(hc[b][:, c, :], ps, AF.Silu,
                                         bias=sb_dwb[:, c:c + 1])
            for b in range(BPC):
                newx = []
                for f in range(KD):
                    ps = pt([P, NT], F32, "mm", 5, f"pw2{b}_{f}")
                    for k in range(KC):
                        nc.tensor.matmul(ps, w2[:, k, f * P:(f + 1) * P],
                                         hc[b][:, k, :],
                                         start=(k == 0), stop=(k == KC - 1))
                    nx = st([P, NT], F32R, "xcur", 12, f"xcv{b}_{f}")
                    nc.vector.scalar_tensor_tensor(
                        nx, ps, sb_bpw2[:, f:f + 1], x[b][f].bitcast(F32),
                        OP.add, OP.add)
                    newx.append(nx)
                x[b] = newx

        # ---- post layernorm ----
        def post_stage():
            pq = ln_stats("po")
            for b in range(BPC):
                pb, qb = ln_bcast(b, pq, "po")
                for f in range(KD):
                    tmp = st([P, NT], F32, "t2k", 3, f"pt{b}_{f}")
                    nc.vector.tensor_mul(tmp, x[b][f].bitcast(F32), pb)
                    t2 = st([P, NT], F32, "t2k", 3, f"pt2{b}_{f}")
                    nc.vector.tensor_tensor(t2, tmp, qb, OP.subtract)
                    yt = st([P, NT], F32, "yout", 2, f"y{b}_{f}")
                    nc.vector.tensor_scalar(yt, t2, sb_postg[:, f:f + 1],
                                            sb_postb[:, f:f + 1],
                                            OP.mult, OP.add)
                    nc.sync.dma_start(pr["yO"][b, f * P:(f + 1) * P, :], yt[:])

        def tap(i):
            if not DEBUG_TAPS:
                return
            for b in range(BPC):
                for f in range(KD):
                    nc.sync.dma_start(pr[f"dbg{i}"][b, f * P:(f + 1) * P, :],
                                      x[b][f].bitcast(F32)[:])

        ff_stage("w_ff1_1", "w_ff1_2", sb_bff11, sb_bff12, "f1")
        tap(1)
        attn_stage()
        tap(2)
        conv_stage()
        tap(3)
        ff_stage("w_ff2_1", "w_ff2_2", sb_bff21, sb_bff22, "f2")
        tap(4)
        post_stage()


# --------------------------------------------------------------------------
# host entry point
# --------------------------------------------------------------------------

_NC = None


def _get_nc():
    global _NC
    if _NC is None:
        _NC = _build_nc()
    return _NC


def _shared_maps(p):
    return {
        'w_ff1_1': p['w_ff1_1'],
        'w_ff1_2': p['w_ff1_2'].astype(BFNP),
        'w_ff2_1': p['w_ff2_1'],
        'w_ff2_2': p['w_ff2_2'].astype(BFNP),
        'qkv_w': p['qkv_w'], 'out_w': p['out_w'],
        'r_mat': p['r_mat'].astype(BFNP),
        'pw1_wT': p['pw1_wT'],
        'pw2_wT': p['pw2_wT'].astype(BFNP),
        'qkv_bv': p['qkv_bv'],
        'b_ff1_1': _col(p['b_ff1_1'], KF), 'b_ff1_2': _col(p['b_ff1_2'], KD),
        'b_ff2_1': _col(p['b_ff2_1'], KF), 'b_ff2_2': _col(p['b_ff2_2'], KD),
        'qkv_b': _col(p['qkv_b'][:2 * INNER], 8),
        'b_pw1': _col(p['b_pw1'], 2 * KC),
        'dw_w': np.ascontiguousarray(
            p['dw_w'].reshape(KC, P, KW).transpose(1, 0, 2)),
        'dw_b': _col(p['dw_b'], KC),
        'b_pw2': _col(p['b_pw2'], KD),
        'post_g': _col(p['post_g'], KD), 'post_b': _col(p['post_b'], KD),
    }


def kernel(**inputs):
    p = _host_prepare(inputs)
    x = np.asarray(inputs['x'], np.float32)
    shared = _shared_maps(p)
    in_maps = []
    for c in range(NCORES):
        m = dict(shared)
        xb = x[c * BPC:(c + 1) * BPC]
        m['xT'] = np.ascontiguousarray(xb.transpose(0, 2, 1))
        in_maps.append(m)

    nc = _get_nc()
    res = run_bass_kernel_spmd(nc, in_maps, core_ids=list(range(NCORES)))
    out = np.empty((B, NT, D), np.float32)
    for c in range(NCORES):
        yT = res.results[c]['y']
        out[c * BPC:(c + 1) * BPC] = yT.transpose(0, 2, 1)
    return out

